# revision 1
# baseline (speedup 1.0000x reference)
"""Trainium2 Bass kernel for DenseInterQTripletLoss.

Strategy (8 NeuronCores, row-sharded):
  - Each core owns 512 rows (cells of desc1) per batch (1024 rows total).
  - S = d1^T @ d2 is computed in bf16 on TensorE, accumulated fp32 in PSUM,
    per [128 rows x 512 cols] blocks.  The visibility penalty (-2.5 per
    invisible column) is folded in exactly via a K=1 rank-1 matmul into the
    same PSUM accumulation group.
  - neg = min(sim) = 2 - 2*max(P).  The row max with the 4-neighbor
    exclusion is done by TensorMaskReduce (per-partition index window
    [ul, ul+66) excluded) directly from PSUM - one DVE pass.
  - pos is computed from a host-relayout "quad" table of desc2 (the 4
    bilinear neighbors of each cell concatenated) gathered per row with
    indirect DMA, then dotted with d1 rows on DVE/GPSIMD.
  - Each core returns [128, 2] partial (sum l, sum wv); host combines.
"""

import os
import numpy as np
import ml_dtypes

GS = 8
B = 2
C = 256
HC = WC = 64
FLAT = HC * WC            # 4096
H = W = 512
NCORES = 8
RPC = FLAT // NCORES      # rows per core per batch = 512
NT = RPC // 128           # row tiles per batch per core = 4
NROWT = B * NT            # row tiles per core = 8
BLK = 512
NBLK = FLAT // BLK        # 8
CH = 2                    # c halves of 128
BIG = 5.0
MARGIN = 1.0

BF16 = ml_dtypes.bfloat16

_cache = {}


def _build_bass(disable=()):
    """disable: subset of {'gather','pen','penflat','vis'} for HW bisection."""
    import concourse.bass as bass
    import concourse.mybir as mybir
    import concourse.tile as tile
    from concourse import bacc
    from concourse.bass import IndirectOffsetOnAxis
    from concourse.dve_ops import TENSOR_MASK_REDUCE, TENSOR_TENSOR_REDUCE

    dt = mybir.dt
    f32, bf16, i32, u8 = dt.float32, dt.bfloat16, dt.int32, dt.uint8
    op = mybir.AluOpType
    AX = mybir.AxisListType

    nc = bacc.Bacc(None)

    # ---- DRAM I/O ----
    d1 = nc.declare_dram_parameter("d1", [B, CH, 128, RPC], bf16, isOutput=False)
    d1r = nc.declare_dram_parameter("d1r", [B, RPC, C], bf16, isOutput=False)
    d2 = nc.declare_dram_parameter("d2", [B, CH, 128, FLAT], bf16, isOutput=False)
    d2q = nc.declare_dram_parameter("d2q", [B * FLAT, 4 * C], bf16, isOutput=False)
    visp = nc.declare_dram_parameter("visp", [B, H, W // 8], u8, isOutput=False)
    # packed consts: [0:16]=c16 windows, [16:20]=x, [20:24]=y, [24:33]=H0, [33:42]=H1
    cst = nc.declare_dram_parameter("cst", [128, 42], f32, isOutput=False)
    outp = nc.declare_dram_parameter("out", [128, 2], f32, isOutput=True)

    with tile.TileContext(nc) as tc:
        import contextlib

        ctx = contextlib.ExitStack()
        with ctx:
            singles = ctx.enter_context(tc.tile_pool(name="singles", bufs=1))
            coords = ctx.enter_context(tc.tile_pool(name="coords", bufs=1))
            d1pool = ctx.enter_context(tc.tile_pool(name="d1pool", bufs=8))
            gpool = ctx.enter_context(tc.tile_pool(name="gpool", bufs=8))
            spool = ctx.enter_context(tc.tile_pool(name="spool", bufs=3))
            psum = ctx.enter_context(tc.tile_pool(name="psum", bufs=7, space="PSUM"))
            small = ctx.enter_context(tc.tile_pool(name="small", bufs=4))
            tiny = ctx.enter_context(tc.tile_pool(name="tiny", bufs=8))

            # ---- constant / bulk loads (single DMA for all small consts) ----
            cst_sb = singles.tile([128, 42], f32)
            nc.sync.dma_start(out=cst_sb[:], in_=cst[:, :])
            c16_sb = cst_sb[:, 0:16]
            x_sb = cst_sb[:, 16 : 16 + NT]
            y_sb = cst_sb[:, 16 + NT : 16 + 2 * NT]
            h_sb = [cst_sb[:, 24:33], cst_sb[:, 33:42]]

            ones_bf = singles.tile([1, 128], bf16)
            nc.vector.memset(ones_bf[:], 1.0)

            # d2 resident tiles
            d2_sb = []
            for b in range(B):
                row = []
                for h in range(CH):
                    t = singles.tile([128, FLAT], bf16, tag=f"d2_{b}_{h}")
                    nc.sync.dma_start(out=t[:], in_=d2[b, h, :, :])
                    row.append(t)
                d2_sb.append(row)

            # ---- visibility -> penalty row (-2.5 per invisible cell) ----
            penrow = []
            for b in range(B):
                vl = singles.tile([64, 512], u8, tag=f"vl{b}")
                nc.sync.dma_start(
                    out=vl[:],
                    in_=visp[b, :, :].rearrange("(gy r) gx -> gy (r gx)", r=8),
                )
                cs = small.tile([64, 64], f32, tag="cs")
                nc.vector.tensor_reduce(
                    out=cs[:],
                    in_=vl[:, :].rearrange("p (r gx) -> p gx r", r=8),
                    axis=AX.X,
                    op=op.add,
                )
                # visible iff sum of 8 packed bytes == 8*255
                cv = small.tile([64, 64], f32, tag="cv")
                nc.vector.tensor_scalar(
                    out=cv[:], in0=cs[:], scalar1=2040.0, scalar2=None, op0=op.is_equal
                )
                pb = small.tile([64, 64], bf16, tag="pb")
                # pen = 2.5*cv - 2.5  (0 if visible, -2.5 if not)
                nc.vector.tensor_scalar(
                    out=pb[:], in0=cv[:], scalar1=2.5, scalar2=-2.5,
                    op0=op.mult, op1=op.add,
                )
                pr = singles.tile([1, FLAT], bf16, tag=f"pr{b}")
                if "penflat" in disable:
                    nc.vector.memset(pr[0:1, :], 0.0)
                else:
                    nc.sync.dma_start(out=pr[0:1, :], in_=pb[:, :])
                penrow.append(pr)

            # ---- coordinate pipeline, per batch, [128, NT] ----
            # produces: wv, w00,w01,w10,w11 (f32), idx (i32), ul (f32)
            wv_b, wts_b, idx_b, ul_b = [], [], [], []
            g = nc.gpsimd
            for b in range(B):
                hb = h_sb[b]

                def hcol(i):
                    return hb[:, i : i + 1]

                wx = coords.tile([128, NT], f32, tag=f"wx{b}")
                wy = coords.tile([128, NT], f32, tag=f"wy{b}")
                wz = coords.tile([128, NT], f32, tag=f"wz{b}")
                v = nc.vector
                v.tensor_scalar(out=wx[:], in0=x_sb[:], scalar1=hcol(0),
                                scalar2=hcol(2), op0=op.mult, op1=op.add)
                v.scalar_tensor_tensor(out=wx[:], in0=y_sb[:], scalar=hcol(1),
                                       in1=wx[:], op0=op.mult, op1=op.add)
                v.tensor_scalar(out=wy[:], in0=x_sb[:], scalar1=hcol(3),
                                scalar2=hcol(5), op0=op.mult, op1=op.add)
                v.scalar_tensor_tensor(out=wy[:], in0=y_sb[:], scalar=hcol(4),
                                       in1=wy[:], op0=op.mult, op1=op.add)
                v.tensor_scalar(out=wz[:], in0=x_sb[:], scalar1=hcol(6),
                                scalar2=hcol(8), op0=op.mult, op1=op.add)
                v.scalar_tensor_tensor(out=wz[:], in0=y_sb[:], scalar=hcol(7),
                                       in1=wz[:], op0=op.mult, op1=op.add)
                v.tensor_scalar(out=wz[:], in0=wz[:], scalar1=1e-8, scalar2=None,
                                op0=op.add)
                rz = coords.tile([128, NT], f32, tag=f"rz{b}")
                nc.vector.reciprocal(out=rz[:], in_=wz[:])
                xw = coords.tile([128, NT], f32, tag=f"xw{b}")
                yw = coords.tile([128, NT], f32, tag=f"yw{b}")
                nc.vector.tensor_tensor(out=xw[:], in0=wx[:], in1=rz[:], op=op.mult)
                nc.vector.tensor_tensor(out=yw[:], in0=wy[:], in1=rz[:], op=op.mult)

                # wv = (yw in [0,512)) & (xw in [0,512))
                wv = coords.tile([128, NT], f32, tag=f"wv{b}")
                nc.vector.tensor_scalar(out=wv[:], in0=xw[:], scalar1=0.0, scalar2=None,
                                op0=op.is_ge)
                nc.vector.scalar_tensor_tensor(out=wv[:], in0=xw[:], scalar=512.0,
                                       in1=wv[:], op0=op.is_lt, op1=op.mult)
                nc.vector.scalar_tensor_tensor(out=wv[:], in0=yw[:], scalar=0.0,
                                       in1=wv[:], op0=op.is_ge, op1=op.mult)
                nc.vector.scalar_tensor_tensor(out=wv[:], in0=yw[:], scalar=512.0,
                                       in1=wv[:], op0=op.is_lt, op1=op.mult)

                # descriptor-space coords
                vy = coords.tile([128, NT], f32, tag=f"vy{b}")
                vx = coords.tile([128, NT], f32, tag=f"vx{b}")
                nc.vector.tensor_scalar(out=vy[:], in0=yw[:], scalar1=0.125, scalar2=None,
                                op0=op.mult)
                nc.vector.tensor_scalar(out=vx[:], in0=xw[:], scalar1=0.125, scalar2=None,
                                op0=op.mult)

                # bilinear base indices: yd = clip(vy, 0, 63); y0 = trunc(yd)
                yd = coords.tile([128, NT], f32, tag=f"yd{b}")
                xd = coords.tile([128, NT], f32, tag=f"xd{b}")
                nc.vector.tensor_scalar(out=yd[:], in0=vy[:], scalar1=0.0, scalar2=63.0,
                                op0=op.max, op1=op.min)
                nc.vector.tensor_scalar(out=xd[:], in0=vx[:], scalar1=0.0, scalar2=63.0,
                                op0=op.max, op1=op.min)
                # floor robust to trunc-or-round f32->i32 conversion:
                #   c = cvt(x); floor = c - (c > x)
                ti = coords.tile([128, NT], i32, tag=f"ti{b}")
                y0 = coords.tile([128, NT], f32, tag=f"y0{b}")
                x0 = coords.tile([128, NT], f32, tag=f"x0{b}")
                ce = coords.tile([128, NT], f32, tag=f"ce{b}")
                nc.vector.tensor_copy(out=ti[:], in_=yd[:])
                nc.vector.tensor_copy(out=y0[:], in_=ti[:])
                nc.vector.tensor_tensor(out=ce[:], in0=y0[:], in1=yd[:], op=op.is_gt)
                nc.vector.tensor_tensor(out=y0[:], in0=y0[:], in1=ce[:], op=op.subtract)
                nc.vector.tensor_copy(out=ti[:], in_=xd[:])
                nc.vector.tensor_copy(out=x0[:], in_=ti[:])
                nc.vector.tensor_tensor(out=ce[:], in0=x0[:], in1=xd[:], op=op.is_gt)
                nc.vector.tensor_tensor(out=x0[:], in0=x0[:], in1=ce[:], op=op.subtract)
                fy = coords.tile([128, NT], f32, tag=f"fy{b}")
                fx = coords.tile([128, NT], f32, tag=f"fx{b}")
                nc.vector.tensor_tensor(out=fy[:], in0=yd[:], in1=y0[:], op=op.subtract)
                nc.vector.tensor_tensor(out=fx[:], in0=xd[:], in1=x0[:], op=op.subtract)
                ofy = coords.tile([128, NT], f32, tag=f"ofy{b}")
                ofx = coords.tile([128, NT], f32, tag=f"ofx{b}")
                nc.vector.tensor_scalar(out=ofy[:], in0=fy[:], scalar1=-1.0, scalar2=1.0,
                                op0=op.mult, op1=op.add)
                nc.vector.tensor_scalar(out=ofx[:], in0=fx[:], scalar1=-1.0, scalar2=1.0,
                                op0=op.mult, op1=op.add)
                w00 = coords.tile([128, NT], f32, tag=f"w00{b}")
                w01 = coords.tile([128, NT], f32, tag=f"w01{b}")
                w10 = coords.tile([128, NT], f32, tag=f"w10{b}")
                w11 = coords.tile([128, NT], f32, tag=f"w11{b}")
                nc.vector.tensor_tensor(out=w00[:], in0=ofy[:], in1=ofx[:], op=op.mult)
                nc.vector.tensor_tensor(out=w01[:], in0=ofy[:], in1=fx[:], op=op.mult)
                nc.vector.tensor_tensor(out=w10[:], in0=fy[:], in1=ofx[:], op=op.mult)
                nc.vector.tensor_tensor(out=w11[:], in0=fy[:], in1=fx[:], op=op.mult)

                # gather index = y0*64 + x0 + 4096*b  (int32)
                idf = coords.tile([128, NT], f32, tag=f"idf{b}")
                nc.vector.tensor_scalar(out=idf[:], in0=y0[:], scalar1=64.0,
                                scalar2=float(FLAT * b), op0=op.mult, op1=op.add)
                nc.vector.tensor_tensor(out=idf[:], in0=idf[:], in1=x0[:], op=op.add)
                idx = coords.tile([128, NT], i32, tag=f"idx{b}")
                nc.vector.tensor_copy(out=idx[:], in_=idf[:])

                # ul = 64*jy + jx;  j = clamp(ceil(v)-1, 0, 63)
                #   ceil(v)-1 = trunc(v) - (v == trunc(v))   (for v > 0; clamps fix v<=0)
                ul = coords.tile([128, NT], f32, tag=f"ul{b}")
                jt = coords.tile([128, NT], f32, tag=f"jt{b}")
                je = coords.tile([128, NT], f32, tag=f"je{b}")
                # jy:  ceil(v)-1 = floor(v) - (v == floor(v))
                nc.vector.tensor_copy(out=ti[:], in_=vy[:])
                nc.vector.tensor_copy(out=jt[:], in_=ti[:])
                nc.vector.tensor_tensor(out=je[:], in0=jt[:], in1=vy[:], op=op.is_gt)
                nc.vector.tensor_tensor(out=jt[:], in0=jt[:], in1=je[:], op=op.subtract)
                nc.vector.tensor_tensor(out=je[:], in0=vy[:], in1=jt[:], op=op.is_equal)
                nc.vector.tensor_tensor(out=jt[:], in0=jt[:], in1=je[:], op=op.subtract)
                nc.vector.tensor_scalar(out=jt[:], in0=jt[:], scalar1=0.0, scalar2=63.0,
                                op0=op.max, op1=op.min)
                nc.vector.tensor_scalar(out=ul[:], in0=jt[:], scalar1=64.0, scalar2=None,
                                op0=op.mult)
                # jx
                nc.vector.tensor_copy(out=ti[:], in_=vx[:])
                nc.vector.tensor_copy(out=jt[:], in_=ti[:])
                nc.vector.tensor_tensor(out=je[:], in0=jt[:], in1=vx[:], op=op.is_gt)
                nc.vector.tensor_tensor(out=jt[:], in0=jt[:], in1=je[:], op=op.subtract)
                nc.vector.tensor_tensor(out=je[:], in0=vx[:], in1=jt[:], op=op.is_equal)
                nc.vector.tensor_tensor(out=jt[:], in0=jt[:], in1=je[:], op=op.subtract)
                nc.vector.tensor_scalar(out=jt[:], in0=jt[:], scalar1=0.0, scalar2=63.0,
                                op0=op.max, op1=op.min)
                nc.vector.tensor_tensor(out=ul[:], in0=ul[:], in1=jt[:], op=op.add)

                wv_b.append(wv)
                wts_b.append((w00, w01, w10, w11))
                idx_b.append(idx)
                ul_b.append(ul)

            # ---- accumulators ----
            acc_l = singles.tile([128, 1], f32, tag="acc_l")
            nc.vector.memset(acc_l[:], 0.0)

            # ---- main loop over row tiles ----
            for t in range(NROWT):
                b, t4 = t // NT, t % NT

                d1t = [
                    d1pool.tile([128, 128], bf16, tag=f"d1h{h}", name=f"d1h{h}")
                    for h in range(CH)
                ]
                for h in range(CH):
                    nc.sync.dma_start(
                        out=d1t[h][:], in_=d1[b, h, :, t4 * 128 : (t4 + 1) * 128]
                    )
                d1row = d1pool.tile([128, C], bf16, tag="d1row")
                nc.sync.dma_start(
                    out=d1row[:], in_=d1r[b, t4 * 128 : (t4 + 1) * 128, :]
                )

                gath = gpool.tile([128, 4 * C], bf16, tag="gath")
                if "gather" in disable:
                    nc.vector.memset(gath[:], 0.0)
                else:
                    nc.gpsimd.indirect_dma_start(
                        out=gath[:],
                        out_offset=None,
                        in_=d2q[:, :],
                        in_offset=IndirectOffsetOnAxis(
                            ap=idx_b[b][:, t4 : t4 + 1], axis=0
                        ),
                    )

                # mask windows for the 8 column blocks
                wnd = tiny.tile([128, 16], f32, tag="wnd")
                nc.vector.tensor_tensor(
                    out=wnd[:],
                    in0=ul_b[b][:, t4 : t4 + 1].to_broadcast([128, 16]),
                    in1=c16_sb[:],
                    op=op.add,
                )

                bm = tiny.tile([128, NBLK], f32, tag="bm")
                for j in range(NBLK):
                    ps = psum.tile([128, BLK], f32, tag="ps")
                    nc.tensor.matmul(
                        out=ps[:], lhsT=d1t[0][:],
                        rhs=d2_sb[b][0][:, j * BLK : (j + 1) * BLK],
                        start=True, stop=False,
                    )
                    nc.tensor.matmul(
                        out=ps[:], lhsT=d1t[1][:],
                        rhs=d2_sb[b][1][:, j * BLK : (j + 1) * BLK],
                        start=False, stop=("pen" in disable),
                    )
                    if "pen" not in disable:
                        nc.tensor.matmul(
                            out=ps[:], lhsT=ones_bf[:],
                            rhs=penrow[b][0:1, j * BLK : (j + 1) * BLK],
                            start=False, stop=True,
                        )
                    sc = spool.tile([128, BLK], f32, tag="mrout")
                    nc.vector._custom_dve(
                        TENSOR_MASK_REDUCE,
                        out=sc[:],
                        in0=ps[:],
                        in1=wnd[:, 2 * j : 2 * j + 1],          # C3 = window lo
                        s0=wnd[:, 2 * j + 1 : 2 * j + 2],        # C0 = window hi
                        s1=-3.0e38,
                        imm2=1.0,
                        accum_out=bm[:, j : j + 1],
                    )

                maxp = tiny.tile([128, 1], f32, tag="maxp")
                nc.vector.tensor_reduce(
                    out=maxp[:], in_=bm[:], axis=AX.X, op=op.max
                )

                # pos dots: dot_k = sum_c d1row * gath_k
                dots = tiny.tile([128, 4], f32, tag="dots")
                dsc = spool.tile([128, C], bf16, tag="dsc")
                for k in range(4):
                    nc.vector._custom_dve(
                        TENSOR_TENSOR_REDUCE,
                        out=dsc[:],
                        in0=gath[:, k * C : (k + 1) * C],
                        in1=d1row[:],
                        s0=0.0,
                        s1=1.0,
                        accum_out=dots[:, k : k + 1],
                    )
                posd = tiny.tile([128, 1], f32, tag="posd")
                pt = tiny.tile([128, 1], f32, tag="pt")
                w4 = wts_b[b]
                nc.vector.tensor_tensor(out=posd[:], in0=dots[:, 0:1],
                                in1=w4[0][:, t4 : t4 + 1], op=op.mult)
                for k in range(1, 4):
                    nc.vector.tensor_tensor(out=pt[:], in0=dots[:, k : k + 1],
                                    in1=w4[k][:, t4 : t4 + 1], op=op.mult)
                    nc.vector.tensor_tensor(out=posd[:], in0=posd[:], in1=pt[:], op=op.add)

                # l = relu(2*(maxp - posd) + 1)^2 * wv ; acc_l += l
                tq = tiny.tile([128, 1], f32, tag="tq")
                nc.vector.tensor_tensor(out=tq[:], in0=maxp[:], in1=posd[:], op=op.subtract)
                nc.vector.tensor_scalar(out=tq[:], in0=tq[:], scalar1=2.0, scalar2=1.0,
                                op0=op.mult, op1=op.add)
                nc.vector.tensor_scalar(out=tq[:], in0=tq[:], scalar1=0.0, scalar2=None,
                                op0=op.max)
                lq = tiny.tile([128, 1], f32, tag="lq")
                nc.vector.tensor_tensor(out=lq[:], in0=tq[:], in1=tq[:], op=op.mult)
                nc.vector.tensor_tensor(out=lq[:], in0=lq[:],
                                in1=wv_b[b][:, t4 : t4 + 1], op=op.mult)
                nc.vector.tensor_tensor(out=acc_l[:], in0=acc_l[:], in1=lq[:], op=op.add)

            # ---- wv sum and output ----
            res = small.tile([128, 2], f32, tag="res")
            nc.vector.tensor_copy(out=res[:, 0:1], in_=acc_l[:])
            wvs = small.tile([128, 1], f32, tag="wvs")
            nc.vector.tensor_reduce(out=wvs[:], in_=wv_b[0][:], axis=AX.X, op=op.add)
            nc.vector.tensor_reduce(
                out=res[:, 1:2], in_=wv_b[1][:], axis=AX.X, op=op.add
            )
            nc.vector.tensor_tensor(
                out=res[:, 1:2], in0=res[:, 1:2], in1=wvs[:], op=op.add
            )
            nc.sync.dma_start(out=outp[:, :], in_=res[:])

    nc.compile()
    return nc


def _prep_inputs(desc1, desc2, homo12, w_vis_mask1):
    """Host-side sharding / layout prep. Returns per-core input maps."""
    d1f = desc1.reshape(B, CH, 128, FLAT).astype(BF16)
    d2f = desc2.reshape(B, CH, 128, FLAT).astype(BF16)
    d1rf = desc1.reshape(B, C, FLAT).transpose(0, 2, 1).astype(BF16)  # (B,FLAT,C)

    # quad table: 4 bilinear neighbors of each cell, concatenated
    d2t = desc2.reshape(B, C, FLAT).transpose(0, 2, 1)  # (B, FLAT, C) f32
    m = np.arange(FLAT)
    y0, x0 = m // 64, m % 64
    x1 = np.minimum(x0 + 1, 63)
    y1 = np.minimum(y0 + 1, 63)
    i00 = y0 * 64 + x0
    i01 = y0 * 64 + x1
    i10 = y1 * 64 + x0
    i11 = y1 * 64 + x1
    quad = np.concatenate(
        [d2t[:, i00, :], d2t[:, i01, :], d2t[:, i10, :], d2t[:, i11, :]], axis=2
    )  # (B, FLAT, 4C)
    d2q = quad.reshape(B * FLAT, 4 * C).astype(BF16)

    visp = np.packbits(
        np.ascontiguousarray(w_vis_mask1.reshape(B, H, W)), axis=-1
    )  # (B, H, W//8) u8

    common = {
        "d2": np.ascontiguousarray(d2f),
        "d2q": np.ascontiguousarray(d2q),
        "visp": np.ascontiguousarray(visp),
    }

    in_maps = []
    for k in range(NCORES):
        rows = np.arange(RPC * k, RPC * (k + 1))
        cstp = np.zeros((128, 42), np.float32)
        # window offsets: col 2j = -512j (mask_end base), 2j+1 = 66 - 512j
        for j in range(NBLK):
            cstp[:, 2 * j] = -BLK * j
            cstp[:, 2 * j + 1] = 66.0 - BLK * j
        for t4 in range(NT):
            r = rows[t4 * 128 : (t4 + 1) * 128]
            cstp[:, 16 + t4] = (r % 64) * GS        # x = 8*gx
            cstp[:, 16 + NT + t4] = (r // 64) * GS  # y = 8*gy
        cstp[:, 24:33] = homo12[0].reshape(1, 9)
        cstp[:, 33:42] = homo12[1].reshape(1, 9)
        im = dict(common)
        im["d1"] = np.ascontiguousarray(d1f[:, :, :, RPC * k : RPC * (k + 1)])
        im["d1r"] = np.ascontiguousarray(d1rf[:, RPC * k : RPC * (k + 1), :])
        im["cst"] = cstp
        in_maps.append(im)
    return in_maps


def kernel(desc1, desc2, homo12, w_vis_mask1, score2):
    from concourse.bass_utils import run_bass_kernel_spmd

    if "nc" not in _cache:
        _cache["nc"] = _build_bass()
    nc = _cache["nc"]

    in_maps = _prep_inputs(
        np.asarray(desc1, np.float32),
        np.asarray(desc2, np.float32),
        np.asarray(homo12, np.float32),
        np.asarray(w_vis_mask1),
    )
    res = run_bass_kernel_spmd(nc, in_maps, core_ids=list(range(NCORES)))
    tot = np.zeros(2, np.float64)
    for r in res.results:
        tot += r["out"].astype(np.float64).sum(axis=0)
    return np.float32(tot[0] / tot[1])



# revision 3
# speedup vs baseline: 4.2544x; 4.2544x over previous
"""Trainium2 Bass kernel for DenseInterQTripletLoss (v2).

Device computes ONLY the heavy part: P = d1^T @ d2c (bf16 matmul on
TensorE) and the per-row masked max (4-neighbor window excluded) via one
custom-DVE TENSOR_MASK_REDUCE per 128-row tile, reading the bank-spanning
PSUM row directly.  Everything else runs on the host:

  - coordinate pipeline (warp, bilinear weights, ul, wv) from homo12
  - pos = 2 - 2*dot(d1_n, bilinear(d2)) in f32
  - visibility: invisible d2 columns can never win the min (their +BIG
    penalty keeps them above any visible column's sim), so they are
    REMOVED: d2 columns are compacted to the visible set (order-
    preserving), and the exclusion window [ul, ul+66) is remapped by rank
    into compacted coordinates (stays a contiguous interval).
  - final loss assembly: neg = 2 - 2*maxp, l = relu(pos-neg+1)^2 * wv.

Each core owns 512 rows per batch (8 row tiles of 128); compacted d2 is
resident in SBUF.  The bass program is rebuilt per distinct n_pad
(data-dependent compaction width); the build is cached.
"""

import numpy as np
import ml_dtypes

GS = 8
B = 2
C = 256
HC = WC = 64
FLAT = HC * WC            # 4096
H = W = 512
NCORES = 8
RPC = FLAT // NCORES      # rows per core per batch = 512
NT = RPC // 128           # row tiles per batch per core = 4
NROWT = B * NT            # row tiles per core = 8
CH = 2                    # c halves of 128
BLK = 512                 # max matmul free-dim chunk (one PSUM bank)

BF16 = ml_dtypes.bfloat16

_cache = {}


def _build_bass(n_pad):
    import concourse.mybir as mybir
    import concourse.tile as tile
    from concourse import bacc
    from concourse.dve_ops import TENSOR_MASK_REDUCE

    dt = mybir.dt
    f32, bf16 = dt.float32, dt.bfloat16

    chunks = []
    o = 0
    while o < n_pad:
        w = min(BLK, n_pad - o)
        chunks.append((o, w))
        o += w

    nc = bacc.Bacc(None)

    d1 = nc.declare_dram_parameter("d1", [B, CH, 128, RPC], bf16, isOutput=False)
    d2 = nc.declare_dram_parameter("d2", [B, CH, 128, n_pad], bf16, isOutput=False)
    # per row tile: [lo, hi] exclusion window (compacted coords), f32
    wnd = nc.declare_dram_parameter("wnd", [128, 2 * NROWT], f32, isOutput=False)
    outp = nc.declare_dram_parameter("out", [128, NROWT], f32, isOutput=True)

    with tile.TileContext(nc) as tc:
        import contextlib

        ctx = contextlib.ExitStack()
        with ctx:
            singles = ctx.enter_context(tc.tile_pool(name="singles", bufs=1))
            psum = ctx.enter_context(tc.tile_pool(name="psum", bufs=2, space="PSUM"))
            scratch = ctx.enter_context(tc.tile_pool(name="scratch", bufs=2))

            wnd_sb = singles.tile([128, 2 * NROWT], f32, tag="wnd", name="wnd_sb")
            nc.sync.dma_start(out=wnd_sb[:], in_=wnd[:, :])

            # resident tensors: d2 per (batch, k-half) and d1 likewise
            d2_sb = [[None] * CH for _ in range(B)]
            d1_sb = [[None] * CH for _ in range(B)]
            for b in range(B):
                for h in range(CH):
                    t2 = singles.tile([128, n_pad], bf16,
                                      tag=f"d2_{b}_{h}", name=f"d2_{b}_{h}")
                    nc.sync.dma_start(out=t2[:], in_=d2[b, h, :, :])
                    d2_sb[b][h] = t2
                    t1 = singles.tile([128, RPC], bf16,
                                      tag=f"d1_{b}_{h}", name=f"d1_{b}_{h}")
                    nc.sync.dma_start(out=t1[:], in_=d1[b, h, :, :])
                    d1_sb[b][h] = t1

            res = singles.tile([128, NROWT], f32, tag="res", name="res")

            for t in range(NROWT):
                b, t4 = t // NT, t % NT
                rsl = slice(t4 * 128, (t4 + 1) * 128)

                ps = psum.tile([128, n_pad], f32, tag="ps", name="ps")
                for (o, w) in chunks:
                    csl = slice(o, o + w)
                    nc.tensor.matmul(
                        out=ps[:, csl], lhsT=d1_sb[b][0][:, rsl],
                        rhs=d2_sb[b][0][:, csl], start=True, stop=False,
                    )
                    nc.tensor.matmul(
                        out=ps[:, csl], lhsT=d1_sb[b][1][:, rsl],
                        rhs=d2_sb[b][1][:, csl], start=False, stop=True,
                    )

                sc = scratch.tile([128, n_pad], bf16, tag="sc", name="sc")
                nc.vector._custom_dve(
                    TENSOR_MASK_REDUCE,
                    out=sc[:],
                    in0=ps[:],
                    in1=wnd_sb[:, 2 * t : 2 * t + 1],       # C3 = lo
                    s0=wnd_sb[:, 2 * t + 1 : 2 * t + 2],    # C0 = hi (>lo -> excl)
                    s1=-3.0e38,                             # C1 accum init
                    imm2=1.0,                               # C2 scale
                    accum_out=res[:, t : t + 1],
                )

            nc.sync.dma_start(out=outp[:, :], in_=res[:])

    nc.compile()
    return nc


def _host_precompute(desc1, desc2, homo12, w_vis_mask1):
    """Numpy f32 replication of the reference's coordinate pipeline."""
    f = np.float32
    gy, gx = np.meshgrid(np.arange(HC, dtype=f), np.arange(WC, dtype=f),
                         indexing="ij")
    coo1 = np.stack([gx * GS, gy * GS], -1).reshape(-1, 2)          # (flat,2) x,y
    homog = np.concatenate([coo1, np.ones((FLAT, 1), f)], -1)
    wpts = np.einsum("bij,nj->bni", homo12.astype(f), homog)
    w_coo = wpts[..., :2] / (wpts[..., 2:3] + f(1e-8))
    wx, wy = w_coo[..., 0], w_coo[..., 1]

    wv = ((wx >= 0) & (wx < H) & (wy >= 0) & (wy < W)).astype(np.float64)

    d2t = desc2.reshape(B, C, FLAT).transpose(0, 2, 1).astype(f)    # (b,flat,c)
    y = np.clip(wy / GS, 0.0, HC - 1.0)
    x = np.clip(wx / GS, 0.0, WC - 1.0)
    y0 = np.floor(y); x0 = np.floor(x)
    fy = (y - y0)[..., None]; fx = (x - x0)[..., None]
    y0i = y0.astype(np.int32); x0i = x0.astype(np.int32)
    y1i = np.minimum(y0i + 1, HC - 1); x1i = np.minimum(x0i + 1, WC - 1)
    bi = np.arange(B)[:, None]
    v00 = d2t[bi, y0i * WC + x0i]; v01 = d2t[bi, y0i * WC + x1i]
    v10 = d2t[bi, y1i * WC + x0i]; v11 = d2t[bi, y1i * WC + x1i]
    wdesc = (v00 * (1 - fy) * (1 - fx) + v01 * (1 - fy) * fx
             + v10 * fy * (1 - fx) + v11 * fy * fx)

    d1f = desc1.reshape(B, C, FLAT).transpose(0, 2, 1).astype(f)
    pos = 2.0 - 2.0 * np.einsum("bnc,bnc->bn", d1f, wdesc)

    jy = np.clip(np.ceil(wy / GS) - 1, 0, HC - 1)
    jx = np.clip(np.ceil(wx / GS) - 1, 0, WC - 1)
    ul = (jy * WC + jx).astype(np.int64)

    vis = w_vis_mask1.reshape(B, HC, GS, WC, GS).all(axis=(2, 4)).reshape(B, FLAT)
    return wv, pos, ul, vis


def _prep(desc1, desc2, homo12, w_vis_mask1):
    wv, pos, ul, vis = _host_precompute(desc1, desc2, homo12, w_vis_mask1)

    # ---- column compaction (multiple of 128, >= max visible count + 1) ----
    nvis = vis.sum(axis=1).astype(np.int64)
    n_max = int(nvis.max())
    n_pad = min(FLAT, -(-(n_max + 1) // 128) * 128)
    n_pad = max(n_pad, 128)

    d2t = desc2.reshape(B, C, FLAT).astype(np.float32)
    d2c = np.zeros((B, C, n_pad), np.float32)
    lo_c = np.empty((B, FLAT), np.int64)
    hi_c = np.empty((B, FLAT), np.int64)
    for b in range(B):
        vb = np.where(vis[b])[0]
        nb = len(vb)
        d2c[b, :, :nb] = d2t[b][:, vb]
        # rank[i] = number of visible indices < i, rank[FLAT] = nb
        rank = np.zeros(FLAT + 1, np.int64)
        np.cumsum(vis[b].astype(np.int64), out=rank[1:])
        lo = rank[ul[b]]
        hi = rank[np.minimum(ul[b] + 66, FLAT)]
        empty = lo == hi
        lo = np.where(empty, n_pad - 1, lo)
        hi = np.where(empty, n_pad, hi)
        lo_c[b], hi_c[b] = lo, hi

    d2q = np.ascontiguousarray(d2c.reshape(B, CH, 128, n_pad)).astype(BF16)
    d1q = desc1.reshape(B, CH, 128, FLAT).astype(BF16)

    in_maps = []
    for k in range(NCORES):
        rsl = slice(RPC * k, RPC * (k + 1))
        im = {
            "d2": d2q,
            "d1": np.ascontiguousarray(d1q[:, :, :, rsl]),
        }
        wndc = np.zeros((128, 2 * NROWT), np.float32)
        for t in range(NROWT):
            b, t4 = t // NT, t % NT
            rows = np.arange(RPC * k + t4 * 128, RPC * k + (t4 + 1) * 128)
            wndc[:, 2 * t] = lo_c[b][rows]
            wndc[:, 2 * t + 1] = hi_c[b][rows]
        im["wnd"] = wndc
        in_maps.append(im)
    return in_maps, wv, pos, n_pad


def kernel(desc1, desc2, homo12, w_vis_mask1, score2):
    from concourse.bass_utils import run_bass_kernel_spmd

    desc1 = np.asarray(desc1, np.float32)
    desc2 = np.asarray(desc2, np.float32)
    homo12 = np.asarray(homo12, np.float32)
    w_vis_mask1 = np.asarray(w_vis_mask1)

    in_maps, wv, pos, n_pad = _prep(desc1, desc2, homo12, w_vis_mask1)

    if n_pad not in _cache:
        _cache[n_pad] = _build_bass(n_pad)
    nc = _cache[n_pad]

    res = run_bass_kernel_spmd(nc, in_maps, core_ids=list(range(NCORES)))

    maxp = np.empty((B, FLAT), np.float64)
    for k, r in enumerate(res.results):
        m = r["out"].astype(np.float64)          # [128, NROWT]
        for t in range(NROWT):
            b, t4 = t // NT, t % NT
            rows = slice(RPC * k + t4 * 128, RPC * k + (t4 + 1) * 128)
            maxp[b, rows] = m[:, t]

    neg = 2.0 - 2.0 * maxp
    l = np.maximum(pos - neg + 1.0, 0.0) ** 2 * wv
    return np.float32(l.sum() / wv.sum())
